# revision 4
# baseline (speedup 1.0000x reference)
"""Trainium2 Bass kernel for LogSpaceMinGRU.

Math: the reference computes, per (batch, channel), a log-space Heinsen scan:
    hg = x @ W.T ; hidden, gate = split(hg)
    log_h = cumulative-logsumexp formulation of  h_t = (1-z_t) h_{t-1} + z_t g(hidden_t)
    out = exp(log_h)
with z = sigmoid(gate), g(x) = relu(x)+0.5 (x>=0) | sigmoid(x) (x<0).

The log-space form exists only for numerical stability.  In linear space the
recurrence h_t = c_t*h_{t-1} + v_t (c = sigmoid(-gate) in (0,1), v = z*g >= 0)
is a convex-combination update, perfectly stable in f32, and maps onto the
TRN2 DVE `tensor_tensor_scan` instruction (state = d0*state op d1 along the
free dim, fp32 internal state regardless of operand dtype).

Sharding over 8 cores: batch (4) x output-feature-half (2).  Each core
computes, for one batch b and one 512-wide feature slice:
    hg_slice = x[b] @ W_slice.T  -> [4096, 1024] (512 hidden | 512 gate)
    h = scan(...)                -> [512, 4096] (channel-major)
The host pre-transposes x[b] to [d, s] and post-transposes the channel-major
output back, so the device never pays for transposes.

Matmul dtype: fp16 (PE 1 cycle/row — the fp32r rate; stream floor 109 us).
fp8e4m3 + DoubleRow was measured on this hardware at only ~2x fp16 rate per
product (cost model claims 4x), so every fp8 scheme that passes the accuracy
gate (>= 3 compensated products; pure e4m3 gives 4e-2 rel err) costs >= fp16.

Elementwise pipeline (per [128,512] psum half; no Pool engine — Pool cannot
read PSUM and runs ALU ops at 0.42-0.6 efficiency):
    ACT : th = tanh(psh/2)             [= 2*(sigmoid(hidden)-1/2)]
          cc = sigmoid(-psg)           [= c]
    DVE : m  = (th*0.5) max psh        [= g - 1/2; g = max(hid+.5, sig(hid))]
          mv = (cc-1)*m                [= -(1-c)(g-1/2)]
          u  = scan(cc*state - mv)     [= h - 1/2, init -1/2, chained halves]
The +0.5 state shift makes g's "+0.5" vanish from the recurrence; the host
adds it back for free during unshard.  All elementwise tensors are fp16
(2x DVE throughput; scan state stays fp32); the output is stored fp16.

Scheduling:
 - W is loaded via the ACT HWDGE queue, x / h via the SP queue (parallel).
 - Cold start: the first quarter runs k-outer across all 8 PSUM banks with
   W and the first-half x tiles interleaved across both DMA queues, so the
   PE starts ~1.5us in instead of waiting for the full 4 MiB of W+x.
 - Steady state is PE-bound; measured marginal ~126 us/rep vs ~148 us/rep
   for the previous psplit baseline under the same differential benchmark
   (device-resident inputs, queued async dispatch, min statistics).
"""

import sys

sys.path.insert(0, "/opt/trn_rl_repo")

import numpy as np

_B, _S, _D = 4, 4096, 1024
_CH = 512          # channels per core (feature slice)
_Q = 1024          # sequence chunk ("quarter" of S)
_NQ = _S // _Q     # 4
_NK = _D // 128    # 8 contraction tiles
_NP = _CH // 128   # 4 channel tiles

_programs = {}


def _build_program(reps=1):
    import concourse.bass as bass  # noqa: F401  (registers engine classes)
    import concourse.tile as tile
    from concourse import bacc, mybir

    f32 = mybir.dt.float32
    f16 = mybir.dt.float16
    AF = mybir.ActivationFunctionType
    OP = mybir.AluOpType

    nc = bacc.Bacc("TRN2", target_bir_lowering=False, debug=False)
    x_d = nc.dram_tensor("x", [_D, _S], f16, kind="ExternalInput").ap()
    w_d = nc.dram_tensor("w", [_D, 2 * _CH], f16, kind="ExternalInput").ap()
    h_d = nc.dram_tensor("h", [_CH, _S], f16, kind="ExternalOutput").ap()

    with tile.TileContext(nc) as tc:
        with (
            tc.tile_pool(name="wp", bufs=1) as wp,
            tc.tile_pool(name="xp", bufs=3) as xp,
            tc.tile_pool(name="ps", bufs=2, space="PSUM") as ps,
            tc.tile_pool(name="sb", bufs=4) as sb,
            tc.tile_pool(name="hp", bufs=2) as hp,
        ):
            wt = [wp.tile([128, 2 * _CH], f16, tag=f"w{k}", name=f"w{k}")
                  for k in range(_NK)]
            # Cold start: interleave W and q0's first-half x tiles across
            # both HWDGE queues (ACT + SP) so the first matmul can issue
            # ~1.5us in while the rest stream.
            xc = [[xp.tile([128, 512], f16, tag=f"xc{k}_{h}",
                           name=f"xc{k}_{h}")
                   for h in range(2)] for k in range(_NK)]
            for k in range(_NK):
                qa = (k % 2 == 0)
                w_q = nc.scalar.dma_start if qa else nc.sync.dma_start
                x_q = nc.sync.dma_start if qa else nc.scalar.dma_start
                w_q(wt[k][:], w_d[k * 128:(k + 1) * 128, :])
                x_q(xc[k][0][:], x_d[k * 128:(k + 1) * 128, 0:512])
            for k in range(_NK):
                nc.sync.dma_start(
                    xc[k][1][:], x_d[k * 128:(k + 1) * 128, 512:1024])

            hprev = [None] * _NP
            ewt = {}

            def alloc_ew(p):
                ewt[p] = dict(
                    th=sb.tile([128, _Q], f16, tag="th", name="th"),
                    cc=sb.tile([128, _Q], f16, tag="c", name="cc"),
                    m=sb.tile([128, _Q], f16, tag="m", name="m"),
                    mv=sb.tile([128, _Q], f16, tag="mv", name="mv"),
                    u=hp.tile([128, _Q], f16, tag=f"h{p}", name=f"u{p}"),
                )

            def consume_half(p, half, psh_t, psg_t, q):
                """ACT/DVE chain for one [128,512] half + chained half-scan."""
                t = ewt[p]
                hs = slice(half * 512, (half + 1) * 512)
                nc.scalar.activation(t["th"][:, hs], psh_t[:], AF.Tanh,
                                     scale=0.5)
                nc.scalar.activation(t["cc"][:, hs], psg_t[:], AF.Sigmoid,
                                     scale=-1.0)
                nc.vector.scalar_tensor_tensor(
                    t["m"][:, hs], t["th"][:, hs], 0.5, psh_t[:],
                    OP.mult, OP.max)
                nc.vector.scalar_tensor_tensor(
                    t["mv"][:, hs], t["cc"][:, hs], 1.0, t["m"][:, hs],
                    OP.subtract, OP.mult)
                init = ((-0.5 if q == 0 else hprev[p][:, _Q - 1:_Q])
                        if half == 0 else
                        t["u"][:, half * 512 - 1:half * 512])
                nc.vector.tensor_tensor_scan(
                    t["u"][:, hs], t["cc"][:, hs], t["mv"][:, hs], init,
                    OP.mult, OP.subtract)
                # per-half output DMA on the ACT HWDGE queue: the half is
                # final right after its scan, and ACT's queue has dispatch
                # headroom while SP (x loads) is near-saturated.  Measured
                # ~7us/rep faster than one SP-queue DMA per quarter.
                nc.scalar.dma_start(
                    h_d[p * 128:(p + 1) * 128,
                        q * _Q + half * 512:q * _Q + (half + 1) * 512],
                    t["u"][:, hs])

            def finish(p, sq):
                hprev[p] = ewt[p]["u"]

            for it in range(_NQ * reps):
                q = it % _NQ
                sq = slice(q * _Q, (q + 1) * _Q)
                if it == 0:
                    # ---- cold first quarter: k-outer over 8 psum banks ----
                    pst = {}
                    for p in range(_NP):
                        alloc_ew(p)
                    for grp, plist in ((0, (0, 1)), (1, (2, 3))):
                        for p in plist:
                            pst[p, 0, 0] = ps.tile([128, 512], f32,
                                                   tag=f"ph{grp}",
                                                   name=f"psh{grp}")
                            pst[p, 1, 0] = ps.tile([128, 512], f32,
                                                   tag=f"pg{grp}",
                                                   name=f"psg{grp}")
                        if grp == 0:
                            # k-outer: each arriving (w[k], x[k]) pair
                            # unlocks 4 matmuls; PE saturates early
                            for k in range(_NK):
                                for p in plist:
                                    for ei, ec in ((0, p), (1, _NP + p)):
                                        nc.tensor.matmul(
                                            pst[p, ei, 0][:],
                                            wt[k][:, ec * 128:(ec + 1) * 128],
                                            xc[k][0][:],
                                            start=(k == 0),
                                            stop=(k == _NK - 1),
                                            skip_group_check=True)
                        else:
                            for p in plist:
                                for ei, ec in ((0, p), (1, _NP + p)):
                                    for k in range(_NK):
                                        nc.tensor.matmul(
                                            pst[p, ei, 0][:],
                                            wt[k][:, ec * 128:(ec + 1) * 128],
                                            xc[k][0][:],
                                            start=(k == 0),
                                            stop=(k == _NK - 1),
                                            skip_group_check=True)
                        for p in plist:
                            consume_half(p, 0, pst[p, 0, 0], pst[p, 1, 0], q)
                    for grp, plist in ((0, (0, 1)), (1, (2, 3))):
                        for p in plist:
                            pst[p, 0, 1] = ps.tile([128, 512], f32,
                                                   tag=f"ph{grp}",
                                                   name=f"psh{grp}")
                            pst[p, 1, 1] = ps.tile([128, 512], f32,
                                                   tag=f"pg{grp}",
                                                   name=f"psg{grp}")
                            for ei, ec in ((0, p), (1, _NP + p)):
                                for k in range(_NK):
                                    nc.tensor.matmul(
                                        pst[p, ei, 1][:],
                                        wt[k][:, ec * 128:(ec + 1) * 128],
                                        xc[k][1][:],
                                        start=(k == 0),
                                        stop=(k == _NK - 1),
                                        skip_group_check=True)
                        for p in plist:
                            consume_half(p, 1, pst[p, 0, 1], pst[p, 1, 1], q)
                            finish(p, sq)
                    continue
                xq = []
                for k in range(_NK):
                    t = xp.tile([128, _Q], f16, tag=f"x{k}", name=f"x{k}")
                    nc.sync.dma_start(t[:], x_d[k * 128:(k + 1) * 128, sq])
                    xq.append(t)
                for p in range(_NP):
                    psh = [ps.tile([128, 512], f32, tag=f"ph{h}",
                                   name=f"psh{h}") for h in range(2)]
                    psg = [ps.tile([128, 512], f32, tag=f"pg{h}",
                                   name=f"psg{h}") for h in range(2)]
                    for ec, pst in ((p, psh), (_NP + p, psg)):
                        wcol = slice(ec * 128, (ec + 1) * 128)
                        for half in range(2):
                            sh2 = slice(half * 512, (half + 1) * 512)
                            for k in range(_NK):
                                nc.tensor.matmul(
                                    pst[half][:],
                                    wt[k][:, wcol],
                                    xq[k][:, sh2],
                                    start=(k == 0),
                                    stop=(k == _NK - 1),
                                    skip_group_check=True,
                                )
                    alloc_ew(p)
                    for half in range(2):
                        consume_half(p, half, psh[half], psg[half], q)
                    finish(p, sq)

    nc.compile()
    return nc


def _get_program(reps=1, **_ignored):
    key = reps
    if key not in _programs:
        _programs[key] = _build_program(reps)
    return _programs[key]


def _shard_inputs(x, W, **_ignored):
    x = np.ascontiguousarray(x, dtype=np.float32)
    W = np.ascontiguousarray(W, dtype=np.float32)
    in_maps = []
    xT = [np.ascontiguousarray(x[b].T.astype(np.float16)) for b in range(_B)]
    for core in range(_B * 2):
        b, f = divmod(core, 2)
        w_slice = np.concatenate(
            [W[f * _CH:(f + 1) * _CH], W[_D + f * _CH:_D + (f + 1) * _CH]],
            axis=0,
        )  # [1024 (e_local), 1024 (d)]
        wT = np.ascontiguousarray(w_slice.T.astype(np.float16))  # [d, e_local]
        in_maps.append({"x": xT[b], "w": wT})
    return in_maps


def _unshard(results):
    out = np.empty((_B, _S, _D), dtype=np.float32)
    for core in range(_B * 2):
        b, f = divmod(core, 2)
        # device returns u = h - 1/2 (fp16, channel-major)
        out[b, :, f * _CH:(f + 1) * _CH] = \
            results[core]["h"].T.astype(np.float32) + 0.5
    return out


def run_sharded(x, W, reps=1, **kwargs):
    """Run the SPMD kernel; returns (output, BassKernelResults)."""
    from concourse.bass_utils import run_bass_kernel_spmd

    kwargs.pop("mm16", None)  # legacy knob, fp16 is the only path now
    nc = _get_program(reps)
    in_maps = _shard_inputs(x, W)
    last_err = None
    for attempt in range(3):
        try:
            res = run_bass_kernel_spmd(nc, in_maps, list(range(_B * 2)),
                                       **kwargs)
            return _unshard(res.results), res
        except Exception as e:  # transient device errors (NRT_EXEC_UNIT_...)
            last_err = e
    raise last_err


def kernel(x, W):
    out, _ = run_sharded(x, W)
    return out
